# revision 36
# baseline (speedup 1.0000x reference)
"""CrossCoderDecoder forward on 8 trn2 NeuronCores.

x[b,l,d] = sum_f f[b,f] * weight[l,f,d] + bias[l,d]
B=32, L=2, F=65536, D=768, fp32.

Sharding: the F (dict) axis is split 8 ways (8192 features per core).
Each core computes its partial [L, B, D] sums; the host sums the 8
partials and adds the bias (the "all-reduce" of the sharding hint,
done host-side since the output is tiny).

Precision/perf scheme: both operands are cast to bf16 and the matmul
runs in a single streaming pass. The contraction length per output is
F=65536 with random-sign terms, so the bf16 rounding noise averages
out to ~1.6e-3 max-relative error on the output — two orders of
magnitude inside the 2e-2 gate. This halves the weight DMA traffic
vs an fp32-accurate hi/lo split (2 B/elem instead of 4), and weight
DMA is the roofline: 24 MiB/core @ ~420 GB/s (measured two-ring
aggregate) ~= 62 us of stream + ~10 us fixed framework pre/postamble.

Weight DMA layout: per (l, chunk of CHUNK_ROWS k-rows) one dma_start
moves a contiguous [P, KO*D] block into SBUF so each partition reads
one contiguous line. The host pre-packs the weights into exactly that
image (a pure reshape of the bf16 cast: k = ch*CHUNK_ROWS + p*KO + o),
and pre-permutes f into fhl[p, j, B] with the matching k order.

The l loop is outermost so l=0's PSUM->SBUF copy and output DMA
overlap with l=1's matmul stream.
"""

import numpy as np
import ml_dtypes

import concourse.bass as bass
import concourse.tile as tile
from concourse import bacc, mybir
from concourse import bass_utils

B, L, F, D = 32, 2, 65536, 768
NCORES = 8
FS = F // NCORES          # 8192 features per core
P = 128
CHUNK_ROWS = 512          # k-rows per weight DMA (786KB transfers;
                          # 384KB ones only sustain ~300 GB/s vs ~418)
CH = FS // CHUNK_ROWS     # chunks per l
KO = CHUNK_ROWS // P      # k-subtiles per chunk
W_BUFS = 24               # weight tile double-buffering depth
# Ring split: chunk g (= l*CH + ch) rides the SP ring when g is odd,
# plus any extras in SYNC_FLIPS; each ring also carries one half of f
# right behind its first weight chunk, keeping the two trains
# byte-symmetric. The PE consumes chunks alternating rings, so a
# phase-lagged ring stalls it at every other chunk.
SYNC_FLIPS = ()
N_RINGS = 2               # 2: sync+scalar (3 rings: gpsimd caps ~110
                          # GB/s and drags the aggregate to ~330)
NSPLITS = ((0, 512), (512, 768))  # PSUM-bank splits of D

_F32 = mybir.dt.float32
_BF16 = mybir.dt.bfloat16
_BF16_NP = ml_dtypes.bfloat16

_cache = {}


def set_tiling(chunk_rows: int, w_bufs: int | None = None, flips=None):
    """Adjust chunking (for tuning sweeps); drops the cached program."""
    global CHUNK_ROWS, CH, KO, W_BUFS, SYNC_FLIPS
    CHUNK_ROWS = chunk_rows
    CH = FS // CHUNK_ROWS
    KO = CHUNK_ROWS // P
    if w_bufs is not None:
        W_BUFS = w_bufs
    if flips is not None:
        SYNC_FLIPS = tuple(flips)
    _cache.clear()


def _build():
    """Build + schedule the (per-core identical) Bass program once."""
    nc = bacc.Bacc("TRN2", target_bir_lowering=False, debug=False)

    fhl = nc.dram_tensor("fhl", [P, CH * KO, B], _BF16, kind="ExternalInput").ap()
    w = nc.dram_tensor("w", [L, CH, P, KO, D], _BF16, kind="ExternalInput").ap()
    out = nc.dram_tensor("out", [L, B, D], _F32, kind="ExternalOutput").ap()

    with tile.TileContext(nc) as tc:
        with (
            tc.tile_pool(name="fpool", bufs=1) as fpool,
            tc.tile_pool(name="wpool", bufs=W_BUFS) as wpool,
            tc.tile_pool(name="opool", bufs=2) as opool,
            tc.tile_pool(name="psum", bufs=1, space="PSUM") as psum,
        ):
            # f is split in half across both HWDGE rings, each half
            # issued right AFTER that ring's first weight chunk: the two
            # rings then carry identical 12.83MB trains (f whole on one
            # ring makes it finish ~4us late, stalling the PE, which
            # consumes chunks in alternating ring order), and the slow
            # cold-start window moves weight bytes instead of f bytes.
            # (gpsimd's queue is not an option: any traffic on it caps
            # ~110 GB/s and suppresses both HWDGE rings.)
            f_sb = fpool.tile([P, CH * KO, B], _BF16)

            ps = [
                [
                    psum.tile([B, n1 - n0], _F32, name=f"ps_{l}_{i}")
                    for i, (n0, n1) in enumerate(NSPLITS)
                ]
                for l in range(L)
            ]
            if N_RINGS == 3:
                ring_of = lambda g: (nc.scalar, nc.sync, nc.gpsimd)[g % 3]
            else:
                # ACT leads (it ramps slower), SP carries f.
                ring_of = lambda g: (
                    nc.sync if (g % 2 == 1 or g in SYNC_FLIPS) else nc.scalar
                )
            out_dmas = []
            for l in range(L):
                for ch in range(CH):
                    g = l * CH + ch
                    wt = wpool.tile([P, KO, D], _BF16)
                    dma_eng = ring_of(g)
                    if g >= L * CH - N_RINGS:
                        # last chunk per ring: two half transfers, so the
                        # rings co-finish tightly (a solo draining ring
                        # runs at ~half rate) and the PE starts earlier
                        h = KO // 2
                        dma_eng.dma_start(wt[:, :h], w[l, ch, :, :h])
                        dma_eng.dma_start(wt[:, h:], w[l, ch, :, h:])
                    else:
                        dma_eng.dma_start(wt[:], w[l, ch])
                    if g < 2:
                        # half A (j < CH*KO/2, needed by the first
                        # matmuls) behind the leading ring's first chunk
                        fh = CH * KO // 2
                        dma_eng.dma_start(
                            f_sb[:, g * fh : (g + 1) * fh],
                            fhl[:, g * fh : (g + 1) * fh],
                        )
                    for o in range(KO):
                        j = ch * KO + o
                        for i, (n0, n1) in enumerate(NSPLITS):
                            nc.tensor.matmul(
                                ps[l][i][:],
                                f_sb[:, j, :],
                                wt[:, o, n0:n1],
                                start=j == 0,
                                stop=j == CH * KO - 1,
                            )
                # l's accumulation just closed: drain it to SBUF/HBM while
                # the next l's matmul stream runs.
                # The out DMA rides the (otherwise idle) gpsimd queue: an
                # out DMA issued from a weight-ring engine blocks that
                # ring's remaining weight issues until this l's matmuls
                # finish (only gpsimd/SP/ACT can initiate DMAs).
                # For l=1 the drain is the critical tail: split the copy
                # across vector + scalar (the ACT ring has no weight
                # issues left by then; for l=0 it still does, and a PSUM
                # read would stall them behind l=0's matmuls).
                out_sb = opool.tile([B, D], _F32)
                for i, (n0, n1) in enumerate(NSPLITS):
                    if l == L - 1 and i == 1:
                        nc.scalar.copy(out=out_sb[:, n0:n1], in_=ps[l][i][:])
                    else:
                        nc.vector.tensor_copy(out=out_sb[:, n0:n1], in_=ps[l][i][:])
                # defer the out DMAs to after every weight issue: a DMA
                # whose semaphore wait references this l's matmuls would
                # block the issuing engine's remaining weight issues.
                out_dmas.append((out[l], out_sb))
            # out[0] rides the idle gpsimd queue mid-kernel (tiny, no
            # ring suppression); out[1] is the critical tail — the ACT
            # ring is empty by then and ~2x faster than gpsimd's queue.
            nc.gpsimd.dma_start(out_dmas[0][0], out_dmas[0][1][:])
            nc.scalar.dma_start(out_dmas[1][0], out_dmas[1][1][:])

    nc.compile()
    return nc


def _prep_f(f_core: np.ndarray) -> np.ndarray:
    """f_core [B, FS] -> fhl [P, CH*KO, B] bf16 matching the kernel's
    k order (k = ch*CHUNK_ROWS + p*KO + o at fhl[p, ch*KO + o])."""
    ft = f_core.T.astype(_BF16_NP)                     # [FS, B]
    ft = ft.reshape(CH, P, KO, B).transpose(1, 0, 2, 3)
    return np.ascontiguousarray(ft.reshape(P, CH * KO, B))


def _prep_w(w_core: np.ndarray) -> np.ndarray:
    """w_core [L, FS, D] -> [L, CH, P, KO, D] bf16 (exact SBUF image)."""
    return np.ascontiguousarray(
        w_core.astype(_BF16_NP).reshape(L, CH, P, KO, D)
    )


def kernel(f: np.ndarray, weight: np.ndarray, bias: np.ndarray) -> np.ndarray:
    f = np.asarray(f, dtype=np.float32)
    weight = np.asarray(weight, dtype=np.float32)
    bias = np.asarray(bias, dtype=np.float32)

    if "nc" not in _cache:
        _cache["nc"] = _build()
    nc = _cache["nc"]

    in_maps = []
    for c in range(NCORES):
        sl = slice(c * FS, (c + 1) * FS)
        in_maps.append(
            {
                "fhl": _prep_f(f[:, sl]),
                "w": _prep_w(weight[:, sl, :]),
            }
        )

    res = bass_utils.run_bass_kernel_spmd(nc, in_maps, core_ids=list(range(NCORES)))
    partial = np.stack([r["out"] for r in res.results])  # [NCORES, L, B, D]
    total = partial.sum(axis=0)                          # [L, B, D]
    x = total.transpose(1, 0, 2) + bias[None, :, :]      # [B, L, D]
    return x.astype(np.float32)
